# revision 32
# baseline (speedup 1.0000x reference)
"""VQ codebook assignment + nearest upsample on 8 NeuronCores.

Problem (per domain): given features f [B=4, C=256, H=64, W=128] and
centroids c [K=19, C=256], compute argmin_k ||f[b,:,h,w] - c_k||^2 and
nearest-upsample the [64,128] index map to [512,1024] (8x in each axis).
Two independent domains (cross-assigned centroids) x 4 batches = 8 cores,
one batch-image per core, no cross-core communication.

Per-core pipeline (input-DMA-bound; HWDGE runs ~179 GB/s per queue with
4KB descriptors, ~360 GB/s aggregate -- so both queues carry identical
byte schedules and every descriptor row is 4KB+):
  1. cross[k, px] via float32r matmuls: the tiny centroid block
     [128, 19] per C-half stays stationary, 512-pixel feature chunks
     move at ~2x the plain-fp32 rate. fp32r keeps 11 mantissa bits;
     inputs are pre-rounded on the host (the BIR verifier requires it),
     costing ~4 near-tie argmin flips per domain (rel_err ~1e-2,
     budget 2e-2). The -|c|^2/2 bias must NOT ride in the fp32r tensor
     (the DMA rounds it, perturbing scores by ~0.03); it ships as a
     separate fp32 tensor over the gpsimd SWDGE queue.
  2. Bit-exact ScalarE Copy moves scores PSUM->SBUF. The ACT engine's
     DMA triggers are emitted lazily, interleaved between copies:
     HWDGE triggers are ring-capacity paced, and a block of triggers
     queued up front head-of-line blocks ACT compute for tens of us.
  3. PE transposes [19, 128] score chunks to [128 px, 19] pixel-
     partition layout for full 128-lane DVE reductions.
  4. scores = cross - c2/2 via an exact DVE add; argmax index with
     first-match tie semantics: reduce_max, then (is_ge * -1024 +
     (iota+1024)) reduce_min yields k directly. The argmin chain runs
     one block behind the matmul/transpose stream so the in-order DVE
     queue never stalls waiting on PE.
  5. Upsample tail per 2-block group (16 h-rows = 128 output rows):
     one bf16 PE matmul against a constant replication matrix
     R[w, x] = (x//8 == w) performs the 8x y-replication (partition
     direction, via a column-broadcast copy of the index vector; small
     ints are exact in bf16) and the 8x x-replication (free direction)
     in one shot: out[i, x] = sum_w idx[w, g*16 + i//8] * R[w, x].
     The result lands on all 128 partitions, so the 512KB group store
     reads each SBUF partition once (4KB) instead of re-reading 32
     partitions 8x (the ~5 GB/s per-partition SBUF read limit). DVE
     copies cast the exact fp32 integers to int32; group mm and
     cast+store are staggered across block boundaries to avoid DVE
     head-of-line stalls; the 4 group stores drain after input.
"""

import numpy as np

import concourse.bass as bass
import concourse.mybir as mybir
import concourse.tile as tile
from concourse import bacc
from concourse.bass import ds
from concourse.bass_utils import run_bass_kernel_spmd
from concourse.masks import make_identity

F32 = mybir.dt.float32
R32 = mybir.dt.float32r
BF16 = mybir.dt.bfloat16
I32 = mybir.dt.int32

B = 4
C = 256
H, W = 64, 128
K = 19
HL, WL = 512, 1024
NPIX = H * W          # 8192
RB = 8                # image rows per block
NB = H // RB          # 8 blocks
CH = 512              # matmul moving chunk (pixels)
RPC = CH // W         # image rows per chunk: 4
UP = HL // H          # 8x upsample
BIG = 1024.0
FWC = K + NPIX        # fw columns: [w | pixels]
GH = 8                # h-rows per store group (1 block -> 64 out rows)
NG = H // GH          # 8 store groups

_NC_CACHE = None


def _build_nc():
    nc = bacc.Bacc("TRN2", target_bir_lowering=False, debug=False)

    fw_in = nc.dram_tensor("fw", [C, FWC], R32, kind="ExternalInput")
    bias_in = nc.dram_tensor("bias", [128, K], F32, kind="ExternalInput")
    mask_out = nc.dram_tensor("mask", [HL, WL], I32, kind="ExternalOutput")

    fwv = fw_in.ap().rearrange("(a p) n -> p a n", a=2)       # [128, 2, FWC]
    outg = mask_out.ap().rearrange("(g r) x -> g r x", r=64)  # [8, 64, 1024]

    with tile.TileContext(nc) as tc:
        with (
            tc.tile_pool(name="persist", bufs=1) as pp,
            tc.tile_pool(name="work", bufs=6) as wp,
            tc.tile_pool(name="psA", bufs=4, space="PSUM") as psA,
            tc.tile_pool(name="psB", bufs=2, space="PSUM") as psB,
            tc.tile_pool(name="psR", bufs=2, space="PSUM") as psR,
        ):
            fw01 = pp.tile([128, 2, FWC], R32, tag="fw01")
            bias128 = pp.tile([128, K], F32, tag="bias128")
            ident = pp.tile([K, K], F32, tag="ident")
            iota_i = pp.tile([128, K], I32, tag="iota_i")
            iotaf = pp.tile([128, K], F32, tag="iotaf")
            idxv = pp.tile([128, H], F32, tag="idxv")       # [w, h] argmax k
            repm = pp.tile([128, WL], BF16, tag="repm")     # R[w,x] = x//8==w

            # --- setup (gpsimd/DVE, off the DMA critical path). Order on
            # gpsimd: identity first (transposes need it ~15us), then the
            # SWDGE input triggers, then the repm build. ---
            make_identity(nc, ident)
            nc.gpsimd.iota(
                iota_i, pattern=[[1, K]], base=int(BIG), channel_multiplier=0
            )
            nc.vector.tensor_copy(iotaf, iota_i)

            # --- feature loads: per-C-half chunk-pair slices with 4KB+
            # descriptor rows (2KB rows halve HWDGE throughput); half 0
            # -> SP queue, half 1 -> ACT queue, identical byte schedules
            # (~4.1MB each). SP triggers go up front (SP runs nothing
            # else); ACT triggers are emitted lazily between copies to
            # dodge head-of-line ring stalls. ---
            ld_slices = [ds(0, K + 2 * CH)]
            for j in range(1, NB):
                ld_slices.append(ds(K + 2 * j * CH, 2 * CH))
            for sl in ld_slices:
                nc.sync.dma_start(fw01[:, 0, sl], fwv[:, 0, sl])
            # unrounded -|c|^2/2 (a DMA into the fp32r fw tensor would
            # round it and perturb scores by ~0.03)
            nc.gpsimd.dma_start(bias128, bias_in[:, :])
            # R[w, x] = 1 iff 0 <= x - 8w <= 7 (two affine_select cuts)
            nc.gpsimd.memset(repm, 1.0)
            nc.gpsimd.affine_select(
                out=repm, in_=repm, compare_op=mybir.AluOpType.is_ge,
                fill=0.0, base=UP - 1, pattern=[[-1, WL]],
                channel_multiplier=UP,
            )
            nc.gpsimd.affine_select(
                out=repm, in_=repm, compare_op=mybir.AluOpType.is_ge,
                fill=0.0, base=0, pattern=[[1, WL]],
                channel_multiplier=-UP,
            )

            act_slices = list(ld_slices)
            act_next = [0]

            def emit_act_trigger():
                if act_next[0] < len(act_slices):
                    sl = act_slices[act_next[0]]
                    nc.scalar.dma_start(fw01[:, 1, sl], fwv[:, 1, sl])
                    act_next[0] += 1

            for _ in range(4):
                emit_act_trigger()

            iota_b = iotaf.rearrange("p (o k) -> p o k", o=1).to_broadcast(
                [128, RB, K]
            )
            bias_b = bias128.rearrange("p (o k) -> p o k", o=1).to_broadcast(
                [128, RB, K]
            )

            ps2s = [None] * NB
            pend = {}
            obs = {}

            def argmin_block(blk):
                ps2 = ps2s[blk]
                S = wp.tile([128, RB, K], F32, tag="S")
                nc.vector.tensor_tensor(S, ps2, bias_b, op=mybir.AluOpType.add)
                maxv = wp.tile([128, RB], F32, tag="maxv")
                nc.vector.tensor_reduce(
                    maxv, S, axis=mybir.AxisListType.X, op=mybir.AluOpType.max
                )
                eq = wp.tile([128, RB, K], F32, tag="eq")
                maxv_b = maxv.rearrange("p (t o) -> p t o", o=1).to_broadcast(
                    [128, RB, K]
                )
                nc.vector.tensor_tensor(eq, S, maxv_b, op=mybir.AluOpType.is_ge)
                cand = wp.tile([128, RB, K], F32, tag="cand")
                nc.vector.scalar_tensor_tensor(
                    cand, eq, -BIG, iota_b,
                    op0=mybir.AluOpType.mult, op1=mybir.AluOpType.add,
                )
                nc.vector.tensor_reduce(
                    idxv[:, ds(blk * RB, RB)], cand,
                    axis=mybir.AxisListType.X, op=mybir.AluOpType.min,
                )

            def emit_idxw(g):
                # column-broadcast copy of the index vector (y-rep along
                # the stationary cols; ints <= 18 are bf16-exact)
                hsl = ds(g * GH, GH)
                idxw = wp.tile([128, GH * UP], BF16, tag="idxw")
                nc.vector.tensor_copy(
                    idxw.rearrange("p (h y) -> p h y", y=UP),
                    idxv[:, hsl].rearrange("p (h o) -> p h o", o=1)
                    .to_broadcast([128, GH, UP]),
                )
                pend[g] = idxw

            def emit_group_mm(g):
                # one bf16 PE matmul per x-half does the 8x y-replication
                # and 8x x-replication in one shot
                idxw = pend.pop(g)
                prs = []
                for xh in range(2):
                    pr = psR.tile([GH * UP, WL // 2], F32, tag="pr")
                    nc.tensor.matmul(
                        pr, idxw, repm[:, ds(xh * (WL // 2), WL // 2)],
                        start=True, stop=True,
                    )
                    prs.append(pr)
                pend[g] = prs

            def emit_group_cast(g):
                prs = pend.pop(g)
                ob = wp.tile([GH * UP, WL], I32, tag="ob", bufs=NG)
                for xh in range(2):
                    # DVE copy casts the exact fp32 integers to int32
                    nc.vector.tensor_copy(
                        ob[:, ds(xh * (WL // 2), WL // 2)], prs[xh]
                    )
                obs[g] = ob

            # --- per-block: matmul -> scores -> transpose; the argmin
            # chain runs one block behind, group repl-mms fire mid-block
            # (PE reaches them ~1.3us after emission so the DVE chain
            # they depend on has drained), casts one block after that.
            # Store triggers go at the very END of each queue's program:
            # a DMA trigger blocks its whole queue at the head until the
            # data semaphore clears. ---
            for blk in range(NB):
                ps2 = psB.tile([128, RB, K], F32, tag="ps2")
                ps2s[blk] = ps2
                for half in range(RB // RPC):
                    ch = blk * (RB // RPC) + half
                    colsl = ds(K + ch * CH, CH)
                    ps = psA.tile([K, CH], F32, tag="ps")
                    nc.tensor.matmul(
                        ps, fw01[:, 0, 0:K], fw01[:, 0, colsl],
                        start=True, stop=False,
                    )
                    nc.tensor.matmul(
                        ps, fw01[:, 1, 0:K], fw01[:, 1, colsl],
                        start=False, stop=True,
                    )
                    # plain ScalarE Copy is bit-exact (the Identity-LUT
                    # bias/scale path is not)
                    St = wp.tile([K, CH], F32, tag="St")
                    nc.scalar.copy(St, ps)
                    for r in range(RPC):
                        nc.tensor.transpose(
                            ps2[:, half * RPC + r],
                            St[:, ds(r * W, W)],
                            ident,
                        )
                    if half == 0:
                        emit_act_trigger()
                        if blk >= 2:
                            emit_group_mm(blk - 2)
                if blk >= 1:
                    argmin_block(blk - 1)
                    emit_idxw(blk - 1)
                    if blk >= 2:
                        emit_group_cast(blk - 2)
            emit_group_mm(NB - 2)
            argmin_block(NB - 1)
            emit_idxw(NB - 1)
            emit_group_cast(NB - 2)
            emit_group_mm(NB - 1)
            emit_group_cast(NB - 1)
            for g in (0, 2, 4, 6):
                nc.sync.dma_start(outg[g], obs[g])
            for g in (1, 3, 5, 7):
                nc.scalar.dma_start(outg[g], obs[g])

    nc.compile()
    return nc


def _round_fp32r(a):
    """Round fp32 to fp32r's 11-bit mantissa (round-half-even), matching
    neuronxcc's cast_fp32_to_fp32r bit-exactly."""
    u = np.ascontiguousarray(a, dtype=np.float32).view(np.uint32)
    lsb = (u >> np.uint32(12)) & np.uint32(1)
    r = (u + np.uint32(0x7FF) + lsb) & np.uint32(0xFFFFF000)
    return r.view(np.float32)


def _prep_domain(feature, centroid):
    """Per-core inputs for one domain: 4 batches against one centroid set."""
    c = np.ascontiguousarray(centroid, dtype=np.float32)
    w = _round_fp32r(np.ascontiguousarray(c.T, dtype=np.float32))  # [C, K]
    c2 = np.sum(c.astype(np.float32) ** 2, axis=1)                 # [K]
    bias = np.ascontiguousarray(
        np.tile((np.float32(-0.5) * c2)[None, :], (128, 1)), dtype=np.float32
    )                                                              # [128, K]
    maps = []
    for b in range(B):
        f = _round_fp32r(
            np.asarray(feature[b], dtype=np.float32).reshape(C, NPIX)
        )
        fw = np.ascontiguousarray(np.concatenate([w, f], axis=1))
        maps.append({"fw": fw, "bias": bias})
    return maps


def kernel(
    feature_s2t, feature_target, label_s2t, label_target,
    centroid_s2t, centroid_target,
):
    global _NC_CACHE
    if _NC_CACHE is None:
        _NC_CACHE = _build_nc()
    nc = _NC_CACHE

    # cross assignment: s2t features vs target centroids, and vice versa
    in_maps = _prep_domain(feature_s2t, centroid_target) + _prep_domain(
        feature_target, centroid_s2t
    )
    res = run_bass_kernel_spmd(nc, in_maps, core_ids=list(range(8))).results
    mask_s2t = np.stack([res[i]["mask"] for i in range(B)]).astype(np.int32)
    mask_target = np.stack([res[B + i]["mask"] for i in range(B)]).astype(
        np.int32
    )
    return (mask_s2t, mask_target)


# revision 33
# speedup vs baseline: 1.0745x; 1.0745x over previous
"""VQ codebook assignment + nearest upsample on 8 NeuronCores.

Problem (per domain): given features f [B=4, C=256, H=64, W=128] and
centroids c [K=19, C=256], compute argmin_k ||f[b,:,h,w] - c_k||^2 and
nearest-upsample the [64,128] index map to [512,1024] (8x in each axis).
Two independent domains (cross-assigned centroids) x 4 batches = 8 cores,
one batch-image per core, no cross-core communication.

Per-core pipeline (input-DMA-bound; HWDGE runs ~179 GB/s per queue with
4KB descriptors, ~360 GB/s aggregate -- so both queues carry identical
byte schedules and every descriptor row is 4KB+):
  1. cross[k, px] via float32r matmuls: the tiny centroid block
     [128, 19] per C-half stays stationary, 512-pixel feature chunks
     move at ~2x the plain-fp32 rate. fp32r keeps 11 mantissa bits;
     inputs are pre-rounded on the host (the BIR verifier requires it),
     costing ~4 near-tie argmin flips per domain (rel_err ~1e-2,
     budget 2e-2). The -|c|^2/2 bias must NOT ride in the fp32r tensor
     (the DMA rounds it, perturbing scores by ~0.03); it ships as a
     separate fp32 tensor over the gpsimd SWDGE queue.
  2. Bit-exact ScalarE Copy moves scores PSUM->SBUF. The ACT engine's
     DMA triggers are emitted lazily, interleaved between copies:
     HWDGE triggers are ring-capacity paced, and a block of triggers
     queued up front head-of-line blocks ACT compute for tens of us.
  3. PE transposes [19, 128] score chunks to [128 px, 19] pixel-
     partition layout for full 128-lane DVE reductions.
  4. scores = cross - c2/2 via an exact DVE add; argmax index with
     first-match tie semantics: reduce_max, then (is_ge * -1024 +
     (iota+1024)) reduce_min yields k directly. The argmin chain runs
     one block behind the matmul/transpose stream so the in-order DVE
     queue never stalls waiting on PE.
  5. Upsample tail per 2-block group (16 h-rows = 128 output rows):
     one bf16 PE matmul against a constant replication matrix
     R[w, x] = (x//8 == w) performs the 8x y-replication (partition
     direction, via a column-broadcast copy of the index vector; small
     ints are exact in bf16) and the 8x x-replication (free direction)
     in one shot: out[i, x] = sum_w idx[w, g*16 + i//8] * R[w, x].
     The result lands on all 128 partitions, so the 512KB group store
     reads each SBUF partition once (4KB) instead of re-reading 32
     partitions 8x (the ~5 GB/s per-partition SBUF read limit). DVE
     copies cast the exact fp32 integers to int32; group mm and
     cast+store are staggered across block boundaries to avoid DVE
     head-of-line stalls; the 4 group stores drain after input.
"""

import numpy as np

import concourse.bass as bass
import concourse.mybir as mybir
import concourse.tile as tile
from concourse import bacc
from concourse.bass import ds
from concourse.bass_utils import run_bass_kernel_spmd
from concourse.masks import make_identity

F32 = mybir.dt.float32
R32 = mybir.dt.float32r
BF16 = mybir.dt.bfloat16
I32 = mybir.dt.int32

B = 4
C = 256
H, W = 64, 128
K = 19
HL, WL = 512, 1024
NPIX = H * W          # 8192
RB = 8                # image rows per block
NB = H // RB          # 8 blocks
CH = 512              # matmul moving chunk (pixels)
RPC = CH // W         # image rows per chunk: 4
UP = HL // H          # 8x upsample
BIG = 1024.0
FWC = K + NPIX        # fw columns: [w | pixels]
GH = 16               # h-rows per store group (2 blocks -> 128 out rows)
NG = H // GH          # 4 store groups

_NC_CACHE = None


def _build_nc():
    nc = bacc.Bacc("TRN2", target_bir_lowering=False, debug=False)

    fw_in = nc.dram_tensor("fw", [C, FWC], R32, kind="ExternalInput")
    bias_in = nc.dram_tensor("bias", [128, K], F32, kind="ExternalInput")
    mask_out = nc.dram_tensor("mask", [HL, WL], I32, kind="ExternalOutput")

    fwv = fw_in.ap().rearrange("(a p) n -> p a n", a=2)        # [128, 2, FWC]
    outg = mask_out.ap().rearrange("(g r) x -> g r x", r=128)  # [4, 128, 1024]

    with tile.TileContext(nc) as tc:
        with (
            tc.tile_pool(name="persist", bufs=1) as pp,
            tc.tile_pool(name="work", bufs=6) as wp,
            tc.tile_pool(name="psA", bufs=4, space="PSUM") as psA,
            tc.tile_pool(name="psB", bufs=2, space="PSUM") as psB,
            tc.tile_pool(name="psR", bufs=2, space="PSUM") as psR,
        ):
            fw01 = pp.tile([128, 2, FWC], R32, tag="fw01")
            bias128 = pp.tile([128, K], F32, tag="bias128")
            ident = pp.tile([K, K], F32, tag="ident")
            iota_i = pp.tile([128, K], I32, tag="iota_i")
            iotaf = pp.tile([128, K], F32, tag="iotaf")
            idxv = pp.tile([128, H], F32, tag="idxv")       # [w, h] argmax k
            repm = pp.tile([128, WL], BF16, tag="repm")     # R[w,x] = x//8==w

            # --- setup (gpsimd/DVE, off the DMA critical path). Order on
            # gpsimd: identity first (transposes need it ~15us), then the
            # SWDGE input triggers, then the repm build. ---
            make_identity(nc, ident)
            nc.gpsimd.iota(
                iota_i, pattern=[[1, K]], base=int(BIG), channel_multiplier=0
            )
            nc.vector.tensor_copy(iotaf, iota_i)

            # --- feature loads: per-C-half chunk-pair slices with 4KB+
            # descriptor rows (2KB rows halve HWDGE throughput); half 0
            # -> SP queue, half 1 -> ACT queue, identical byte schedules
            # (~4.1MB each). SP triggers go up front (SP runs nothing
            # else); ACT triggers are emitted lazily between copies to
            # dodge head-of-line ring stalls. ---
            ld_slices = [ds(0, K + 2 * CH)]
            for j in range(1, NB):
                ld_slices.append(ds(K + 2 * j * CH, 2 * CH))
            for sl in ld_slices:
                nc.sync.dma_start(fw01[:, 0, sl], fwv[:, 0, sl])
            # unrounded -|c|^2/2 (a DMA into the fp32r fw tensor would
            # round it and perturb scores by ~0.03)
            nc.gpsimd.dma_start(bias128, bias_in[:, :])
            # R[w, x] = 1 iff 0 <= x - 8w <= 7 (two affine_select cuts)
            nc.gpsimd.memset(repm, 1.0)
            nc.gpsimd.affine_select(
                out=repm, in_=repm, compare_op=mybir.AluOpType.is_ge,
                fill=0.0, base=UP - 1, pattern=[[-1, WL]],
                channel_multiplier=UP,
            )
            nc.gpsimd.affine_select(
                out=repm, in_=repm, compare_op=mybir.AluOpType.is_ge,
                fill=0.0, base=0, pattern=[[1, WL]],
                channel_multiplier=-UP,
            )

            act_slices = list(ld_slices)
            act_next = [0]

            def emit_act_trigger():
                if act_next[0] < len(act_slices):
                    sl = act_slices[act_next[0]]
                    nc.scalar.dma_start(fw01[:, 1, sl], fwv[:, 1, sl])
                    act_next[0] += 1

            for _ in range(4):
                emit_act_trigger()

            iota_b = iotaf.rearrange("p (o k) -> p o k", o=1).to_broadcast(
                [128, RB, K]
            )
            bias_b = bias128.rearrange("p (o k) -> p o k", o=1).to_broadcast(
                [128, RB, K]
            )

            ps2s = [None] * NB
            pend = {}
            obs = {}

            def argmin_block(blk):
                ps2 = ps2s[blk]
                S = wp.tile([128, RB, K], F32, tag="S")
                nc.vector.tensor_tensor(S, ps2, bias_b, op=mybir.AluOpType.add)
                maxv = wp.tile([128, RB], F32, tag="maxv")
                nc.vector.tensor_reduce(
                    maxv, S, axis=mybir.AxisListType.X, op=mybir.AluOpType.max
                )
                eq = wp.tile([128, RB, K], F32, tag="eq")
                maxv_b = maxv.rearrange("p (t o) -> p t o", o=1).to_broadcast(
                    [128, RB, K]
                )
                nc.vector.tensor_tensor(eq, S, maxv_b, op=mybir.AluOpType.is_ge)
                cand = wp.tile([128, RB, K], F32, tag="cand")
                nc.vector.scalar_tensor_tensor(
                    cand, eq, -BIG, iota_b,
                    op0=mybir.AluOpType.mult, op1=mybir.AluOpType.add,
                )
                nc.vector.tensor_reduce(
                    idxv[:, ds(blk * RB, RB)], cand,
                    axis=mybir.AxisListType.X, op=mybir.AluOpType.min,
                )

            def emit_idxw(g):
                # column-broadcast copy of the index vector (y-rep along
                # the stationary cols; ints <= 18 are bf16-exact)
                hsl = ds(g * GH, GH)
                idxw = wp.tile([128, GH * UP], BF16, tag="idxw")
                nc.vector.tensor_copy(
                    idxw.rearrange("p (h y) -> p h y", y=UP),
                    idxv[:, hsl].rearrange("p (h o) -> p h o", o=1)
                    .to_broadcast([128, GH, UP]),
                )
                pend[g] = idxw

            def emit_group_mm(g):
                # one bf16 PE matmul per x-half does the 8x y-replication
                # and 8x x-replication in one shot
                idxw = pend.pop(g)
                prs = []
                for xh in range(2):
                    pr = psR.tile([GH * UP, WL // 2], F32, tag="pr")
                    nc.tensor.matmul(
                        pr, idxw, repm[:, ds(xh * (WL // 2), WL // 2)],
                        start=True, stop=True,
                    )
                    prs.append(pr)
                pend[g] = prs

            def emit_group_cast(g):
                prs = pend.pop(g)
                ob = wp.tile([GH * UP, WL], I32, tag="ob", bufs=NG)
                for xh in range(2):
                    # DVE copy casts the exact fp32 integers to int32
                    nc.vector.tensor_copy(
                        ob[:, ds(xh * (WL // 2), WL // 2)], prs[xh]
                    )
                obs[g] = ob

            # --- per-block: matmul -> scores -> transpose; the argmin
            # chain runs one block behind, group repl-mms fire mid-block
            # (PE reaches them ~1.3us after emission so the DVE chain
            # they depend on has drained), casts one block after that.
            # Store triggers go at the very END of each queue's program:
            # a DMA trigger blocks its whole queue at the head until the
            # data semaphore clears. ---
            for blk in range(NB):
                ps2 = psB.tile([128, RB, K], F32, tag="ps2")
                ps2s[blk] = ps2
                for half in range(RB // RPC):
                    ch = blk * (RB // RPC) + half
                    colsl = ds(K + ch * CH, CH)
                    ps = psA.tile([K, CH], F32, tag="ps")
                    nc.tensor.matmul(
                        ps, fw01[:, 0, 0:K], fw01[:, 0, colsl],
                        start=True, stop=False,
                    )
                    nc.tensor.matmul(
                        ps, fw01[:, 1, 0:K], fw01[:, 1, colsl],
                        start=False, stop=True,
                    )
                    # plain ScalarE Copy is bit-exact (the Identity-LUT
                    # bias/scale path is not)
                    St = wp.tile([K, CH], F32, tag="St")
                    nc.scalar.copy(St, ps)
                    for r in range(RPC):
                        nc.tensor.transpose(
                            ps2[:, half * RPC + r],
                            St[:, ds(r * W, W)],
                            ident,
                        )
                    if half == 0:
                        emit_act_trigger()
                        if blk % 2 == 1 and blk >= 3:
                            emit_group_mm(blk // 2 - 1)
                if blk >= 1:
                    argmin_block(blk - 1)
                    if (blk - 1) % 2 == 1:
                        emit_idxw((blk - 1) // 2)
                    if blk % 2 == 1 and blk >= 3:
                        emit_group_cast(blk // 2 - 1)
            argmin_block(NB - 1)
            emit_idxw(NG - 1)
            emit_group_mm(NG - 1)
            emit_group_cast(NG - 1)
            for g in (0, 2):
                nc.sync.dma_start(outg[g], obs[g])
            for g in (1, 3):
                nc.scalar.dma_start(outg[g], obs[g])

    nc.compile()
    return nc


def _round_fp32r(a):
    """Round fp32 to fp32r's 11-bit mantissa (round-half-even), matching
    neuronxcc's cast_fp32_to_fp32r bit-exactly."""
    u = np.ascontiguousarray(a, dtype=np.float32).view(np.uint32)
    lsb = (u >> np.uint32(12)) & np.uint32(1)
    r = (u + np.uint32(0x7FF) + lsb) & np.uint32(0xFFFFF000)
    return r.view(np.float32)


def _prep_domain(feature, centroid):
    """Per-core inputs for one domain: 4 batches against one centroid set."""
    c = np.ascontiguousarray(centroid, dtype=np.float32)
    w = _round_fp32r(np.ascontiguousarray(c.T, dtype=np.float32))  # [C, K]
    c2 = np.sum(c.astype(np.float32) ** 2, axis=1)                 # [K]
    bias = np.ascontiguousarray(
        np.tile((np.float32(-0.5) * c2)[None, :], (128, 1)), dtype=np.float32
    )                                                              # [128, K]
    maps = []
    for b in range(B):
        f = _round_fp32r(
            np.asarray(feature[b], dtype=np.float32).reshape(C, NPIX)
        )
        fw = np.ascontiguousarray(np.concatenate([w, f], axis=1))
        maps.append({"fw": fw, "bias": bias})
    return maps


def kernel(
    feature_s2t, feature_target, label_s2t, label_target,
    centroid_s2t, centroid_target,
):
    global _NC_CACHE
    if _NC_CACHE is None:
        _NC_CACHE = _build_nc()
    nc = _NC_CACHE

    # cross assignment: s2t features vs target centroids, and vice versa
    in_maps = _prep_domain(feature_s2t, centroid_target) + _prep_domain(
        feature_target, centroid_s2t
    )
    res = run_bass_kernel_spmd(nc, in_maps, core_ids=list(range(8))).results
    mask_s2t = np.stack([res[i]["mask"] for i in range(B)]).astype(np.int32)
    mask_target = np.stack([res[B + i]["mask"] for i in range(B)]).astype(
        np.int32
    )
    return (mask_s2t, mask_target)
